# revision 6
# baseline (speedup 1.0000x reference)
"""Trainium2 Bass kernel for nn_MultiHeadAttention (B=2, S=2048, D=1024, H=16).

Sharding: 8 cores; core c handles batch b=c//4 and the 4 heads
h in [4*(c%4), 4*(c%4)+4). Attention is embarrassingly parallel over (B, H);
the output projection is computed per-core over its head group (partial sums),
and the host sums the 4 partials per batch and adds the output bias.

All matmul operands are fp16 (10 mantissa bits — the same precision class as
tf32/float32r, measured end-to-end rel err ~1.5e-3) with fp32 PSUM
accumulation. fp16 halves every DMA against the serial ~360GB/s DMA-engine
resource and halves SBUF footprints vs fp32r at the same 1 row/cycle PE rate.

Per-core dataflow (contraction dim always on SBUF partitions):
  - host pre-packs q/k/v per batch into the exact per-partition SBUF layouts
    (flat [128, ...] slabs so every DMA descriptor is >=1KB: no
    small-descriptor 2x penalty) and converts to fp16
  - qh^T / kh^T [d, s] computed 2-heads-packed: head A on partitions 0-63,
    head B on 64-127 (lhsT = packed Wq columns, rhs = streamed xT chunks)
  - vh computed in natural [s, d] layout, with a ones-column appended -> the
    AV matmul also yields the softmax denominators
  - scores computed transposed s^T[k, q] so the softmax numerator
    exp(0.125*s + log2*causal) is produced by ScalarE directly in the
    AV-ready layout (k on partitions); no transposes needed anywhere on-chip.
    The reference's "mask" log(tril*1e-9 + 1e-9) is, by softmax shift
    invariance, exactly a x2 weight on the lower triangle.
  - scores/exp run in 4-key-block QUADS: one [128, 4, 512] psum tile per
    (head, quad), one ScalarE exp per quad. The dense sweeps are exp-paced
    (ScalarE is the secondary bottleneck at ~121us vs the PE's ~165us), so
    halving ScalarE's per-instruction PSUM-access overhead is critical.
  - AV: psum[65, 512] accumulates vh_aug.T @ e^T over 16 k-chunks; row 64 is
    the denominator. Normalize via DVE reciprocal + partition-broadcast.
  - out projection: head pairs stacked to K=128 (odd head hopped to
    partitions 64-127 over a SBUF->SBUF DMA); per (s-block, D-chunk) the two
    pair matmuls accumulate in psum; partial [S, D] DMAed out in fp16 on the
    Pool engine's SWDGE queue (keeps the SP queue free for input loads).
  - schedule: every projection matmul (k/v at qc=0, q chunk prefetch,
    out-projection groups) is spread as per-quad fill across BOTH packs'
    sweeps; DMA loads are issued 1-2 quads before their consuming matmuls so
    the in-order PE queue never waits on a same-slot DMA.
  - tail: the last q-chunk's out-projection is split by pack (pack0's half
    runs inside pack1's final sweep into a separate slab the host adds), and
    the pack halves use K=64 matmuls straight off the per-head normalize
    outputs — no partition-stack DMA on the critical path.
"""
import numpy as np
from contextlib import ExitStack

import concourse.bacc as bacc
import concourse.mybir as mybir
import concourse.tile as tile
from concourse.bass_utils import run_bass_kernel_spmd

F32 = mybir.dt.float32
F16 = mybir.dt.float16
AF = mybir.ActivationFunctionType
ALU = mybir.AluOpType

B, S, D, H, PD = 2, 2048, 1024, 16, 64
NCORES = 8
HPC = H * B // NCORES        # 4 heads per core
NPACK = HPC // 2             # 2 head-pairs per core
HPD = HPC * PD               # 256 projected columns per core
SC = 512                     # free-dim chunk (one fp32 psum bank)
NSC = S // SC                # 4
NKB = S // 128               # 16 key blocks / s blocks
NDC = D // 128               # 8 contraction chunks for the projections
NQD = 4                      # key-block quads per s-chunk
LOG2 = float(np.log(2.0))

# fp32 cst blob column layout (per partition)
CST_BQ = 0                   # [2] per-pack bq (per-partition scalars)
CST_BK = CST_BQ + 2          # [2]
CST_BV = CST_BK + 2          # [256] bv broadcast (free-dim layout)
CST_LOG2 = CST_BV + HPD      # [1] log(2) per partition (exp bias)
CST_ZERO = CST_LOG2 + 1      # [1] 0.0 per partition (exp bias)
CST_ONE = CST_ZERO + 1       # [1] 1.0 per partition
CST_COLS = CST_ONE + 1


def _build(causal: bool):
    nc = bacc.Bacc()
    qp = nc.dram_tensor("qp", [128, NSC * NDC * SC], F16, kind="ExternalInput")
    kp = nc.dram_tensor("kp", [128, NSC * NDC * SC], F16, kind="ExternalInput")
    vp = nc.dram_tensor("vp", [128, NKB * NDC * 128], F16,
                        kind="ExternalInput")
    wq = nc.dram_tensor("wq", [128, NDC * HPD], F16, kind="ExternalInput")
    wk = nc.dram_tensor("wk", [128, NDC * HPD], F16, kind="ExternalInput")
    wv = nc.dram_tensor("wv", [128, NDC * HPD], F16, kind="ExternalInput")
    wo = nc.dram_tensor("wo", [128, NPACK * D], F16, kind="ExternalInput")
    cst = nc.dram_tensor("cst", [128, CST_COLS], F32, kind="ExternalInput")
    msk = nc.dram_tensor("msk", [128, NQD * SC], F16, kind="ExternalInput")
    out_d = nc.dram_tensor("out", [S, D], F16, kind="ExternalOutput")
    out2_d = nc.dram_tensor("out2", [SC, D], F16, kind="ExternalOutput")

    mm = nc.tensor.matmul

    with tile.TileContext(nc) as tc, ExitStack() as ctx:
        cpool = ctx.enter_context(tc.tile_pool(name="cpool", bufs=1))
        xpool = ctx.enter_context(tc.tile_pool(name="xpool", bufs=2))
        hpool = ctx.enter_context(tc.tile_pool(name="hpool", bufs=1))
        epool = ctx.enter_context(tc.tile_pool(name="epool", bufs=3))
        opool = ctx.enter_context(tc.tile_pool(name="opool", bufs=2))
        spool = ctx.enter_context(tc.tile_pool(name="spool", bufs=2))
        pspool = ctx.enter_context(tc.tile_pool(name="ps", bufs=2,
                                                space="PSUM"))

        # ---- constants; HWDGE DMAs drain in emission order, so emit in
        # first-use order ----
        wq_t = cpool.tile([128, NDC * HPD], F16)
        nc.sync.dma_start(wq_t[:, 0:NDC * HPD // 2], wq[:, 0:NDC * HPD // 2])
        nc.sync.dma_start(wq_t[:, NDC * HPD // 2:], wq[:, NDC * HPD // 2:])
        cst_t = cpool.tile([128, CST_COLS], F32)
        nc.sync.dma_start(cst_t[:], cst[:])
        ones1 = cpool.tile([1, PD], F16)
        nc.vector.memset(ones1[:], 1.0)
        msk_t = cpool.tile([128, NQD * SC], F16)
        wk_t = cpool.tile([128, NDC * HPD], F16)
        wv_t = cpool.tile([128, NDC * HPD], F16)
        wo_t = cpool.tile([128, NPACK * D], F16)
        wo2_t = cpool.tile([PD, NPACK * D], F16)

        qh = [hpool.tile([128, S], F16, name=f"qh{p}") for p in range(NPACK)]
        kh = [hpool.tile([128, S], F16, name=f"kh{p}") for p in range(NPACK)]
        vh_all = hpool.tile([128, NKB, HPC, PD + 1], F16, name="vh_all")
        nc.vector.tensor_copy(
            vh_all[:, :, :, PD:PD + 1],
            cst_t[:, CST_ONE:CST_ONE + 1].to_broadcast((128, NKB, HPC, 1)))

        def load_x(xdram, sc):
            """DMA one [128, NDC*SC] s-chunk of packed q/k (4 descriptors)."""
            xTc = xpool.tile([128, NDC * SC], F16, tag="xTc", name="xTc",
                             bufs=3)
            w = NDC * SC // 4
            for i in range(4):
                nc.sync.dma_start(
                    xTc[:, i * w:(i + 1) * w],
                    xdram[:, sc * NDC * SC + i * w:sc * NDC * SC +
                          (i + 1) * w])
            return xTc

        def compute_qk(xTc, wtile, htiles, boff, sc):
            """Packed ^T projection matmuls for one loaded s-chunk."""
            for pk in range(NPACK):
                ps = pspool.tile([128, SC], F32, tag="mm", name="ps_qk")
                for dc in range(NDC):
                    mm(ps[:],
                       wtile[:, dc * HPD + pk * 128:
                             dc * HPD + (pk + 1) * 128],
                       xTc[:, dc * SC:(dc + 1) * SC],
                       start=(dc == 0), stop=(dc == NDC - 1))
                nc.vector.tensor_scalar(
                    htiles[pk][:, sc * SC:(sc + 1) * SC], ps[:],
                    cst_t[:, boff + pk: boff + pk + 1], None, ALU.add)

        bv_ap = cst_t[:, CST_BV: CST_BV + HPD].rearrange(
            "p (h d) -> p h d", h=HPC)

        def load_v(sb):
            vsl = xpool.tile([128, NDC * 128], F16, tag="vsl", name="vsl",
                             bufs=4)
            nc.sync.dma_start(
                vsl[:], vp[:, sb * NDC * 128:(sb + 1) * NDC * 128])
            return vsl

        def compute_v(vsl, sb):
            """One 128-row block of the natural-layout v projection."""
            ps = pspool.tile([128, HPD], F32, tag="mm", name="ps_v")
            for dc in range(NDC):
                mm(ps[:], vsl[:, dc * 128:(dc + 1) * 128],
                   wv_t[:, dc * HPD:(dc + 1) * HPD],
                   start=(dc == 0), stop=(dc == NDC - 1))
            nc.vector.tensor_tensor(
                vh_all[:, sb, :, 0:PD],
                ps[:].rearrange("p (h d) -> p h d", h=HPC),
                bv_ap,
                ALU.add)

        def score_exp_quad(qc, pk, hh, qd):
            """Scores^T for FOUR consecutive k-blocks of one head into one
            4-bank psum tile, then a single [128, 4*SC] exp -> et4.

            One exp per quad keeps ScalarE (whose per-op PSUM-access overhead
            would otherwise pace the dense sweeps) at ~59% busy."""
            base = hh * PD
            sps = pspool.tile([128, NQD, SC], F32, tag="sps", name="sps",
                              bufs=1)
            for j in range(NQD):
                kb = NQD * qd + j
                mm(sps[:, j, :],
                   kh[pk][base:base + PD, kb * 128:(kb + 1) * 128],
                   qh[pk][base:base + PD, qc * SC:(qc + 1) * SC])
            et4 = epool.tile([128, NQD, SC], F16, tag=f"e{hh}",
                             name=f"et{hh}", bufs=3)
            if causal and qd == qc:
                # the diagonal quad: its four mask tiles are exactly the
                # msk blob in order (delta 0..3)
                tmp = epool.tile([128, NQD, SC], F16, tag="tmp", name="tmp",
                                 bufs=2)
                nc.vector.scalar_tensor_tensor(
                    tmp[:], sps[:], 0.125,
                    msk_t[:].rearrange("p (j s) -> p j s", j=NQD),
                    ALU.mult, ALU.add)
                nc.scalar.activation(
                    et4[:], tmp[:], AF.Exp,
                    bias=cst_t[:, CST_ZERO:CST_ZERO + 1], scale=1.0)
            else:
                boff = CST_LOG2 if (causal and qd < qc) else CST_ZERO
                nc.scalar.activation(
                    et4[:], sps[:], AF.Exp,
                    bias=cst_t[:, boff:boff + 1], scale=0.125)
            return et4

        def av_mm(av, i4, kb, et):
            mm(av[:], vh_all[:, kb, i4, :], et[:],
               start=(kb == 0), stop=(kb == NKB - 1))

        def attention_pack(qc, pk, tasks=None):
            """4-quad sweep for one pack (2 heads), AV one quad behind.
            Returns the pack's two [65, SC] psum accumulators."""
            avs = [pspool.tile([PD + 1, SC], F32, tag="av", name=f"av{hh}",
                               bufs=2)
                   for hh in range(2)]
            prevs = None
            for qd in range(NQD):
                if tasks is not None:
                    for t in tasks.get(qd, ()):
                        t()
                cur = [score_exp_quad(qc, pk, hh, qd) for hh in range(2)]
                if prevs is not None:
                    for hh in range(2):
                        for j in range(NQD):
                            av_mm(avs[hh], pk * 2 + hh,
                                  NQD * (qd - 1) + j, prevs[hh][:, j, :])
                prevs = cur
            if tasks is not None:
                for t in tasks.get(NQD, ()):
                    t()
            for hh in range(2):
                for j in range(NQD):
                    av_mm(avs[hh], pk * 2 + hh, NKB - NQD + j,
                          prevs[hh][:, j, :])
            return avs

        def normalize_pack(avs, pk, ohs):
            """outh^T = av[0:64] * bcast(1/av[64]) for the pack's 2 heads,
            stacked onto one [128, SC] tile (odd head hops to partitions
            64-127 over a SBUF->SBUF DMA) so the out-projection runs K=128.
            hh0 first: frees the next sweep's first av psum bank sooner."""
            oh = opool.tile([128, SC], F16, tag=f"ohp{pk}", name=f"ohp{pk}")
            ohs.append(oh)
            for hh in (0, 1):
                av = avs[hh]
                rrow = spool.tile([1, SC], F16, tag="rrow", name="rrow")
                with nc.allow_low_precision(
                        reason="fp16 1/denominator: 2^-11 rel, within budget"):
                    nc.vector.reciprocal(rrow[:], av[PD:PD + 1, :])
                rb = spool.tile([PD, SC], F16, tag="rb", name="rb")
                nc.sync.dma_start(
                    rb[:],
                    rrow[0:1, :].rearrange("p (o s) -> p o s",
                                           o=1).broadcast_to((1, PD, SC)))
                if hh == 0:
                    nc.vector.tensor_tensor(oh[0:PD, :], av[0:PD, :], rb[:],
                                            ALU.mult)
                else:
                    stg = spool.tile([PD, SC], F16, tag="stg", name="stg")
                    nc.vector.tensor_tensor(stg[:], av[0:PD, :], rb[:],
                                            ALU.mult)
                    nc.sync.dma_start(oh[PD:128, :], stg[:])

        def normalize_tail(avs):
            """Per-head normalize WITHOUT the partition-stack hop: returns the
            two [PD, SC] tiles. The reciprocal broadcast runs through a K=1
            PE matmul (no DMA-semaphore latency on the end-of-kernel path)."""
            sts = []
            rrows = []
            for hh in (0, 1):
                rrow = spool.tile([1, SC], F16, tag="rrow", name="rrow")
                with nc.allow_low_precision(
                        reason="fp16 1/denominator: 2^-11 rel, within budget"):
                    nc.vector.reciprocal(rrow[:], avs[hh][PD:PD + 1, :])
                rrows.append(rrow)
            for hh in (0, 1):
                bps = pspool.tile([PD, SC], F32, tag="mm", name="bps")
                mm(bps[:], ones1[:], rrows[hh][:])
                rb = spool.tile([PD, SC], F16, tag="rb", name="rb")
                nc.vector.tensor_copy(rb[:], bps[:])
                st = spool.tile([PD, SC], F16, tag="st", name=f"st{hh}",
                                bufs=4)
                nc.vector.tensor_tensor(st[:], avs[hh][0:PD, :], rb[:],
                                        ALU.mult)
                sts.append(st)
            return sts

        def proj_out_group(qc, ohs, g):
            """One stacked-K=128 out-projection group (s-block, D-chunk)."""
            sbl, dc2 = g // 2, g % 2
            sb = qc * 4 + sbl
            pps = pspool.tile([128, SC], F32, tag="mm", name="pps")
            for pr in range(NPACK):
                mm(pps[:],
                   ohs[pr][:, sbl * 128:(sbl + 1) * 128],
                   wo_t[:, pr * D + dc2 * SC:pr * D + (dc2 + 1) * SC],
                   start=(pr == 0), stop=(pr == NPACK - 1))
            oev = opool.tile([128, SC], F16, tag="oev", name="oev", bufs=3)
            nc.vector.tensor_copy(oev[:], pps[:])
            nc.gpsimd.dma_start(
                out_d[sb * 128:(sb + 1) * 128, dc2 * SC:(dc2 + 1) * SC],
                oev[:])

        def proj_out_splitk(sts, pr, g, dest, row_base):
            """One single-pack out-projection group as two K=64 matmuls
            straight off the normalize outputs (no stack hop)."""
            sbl, dc2 = g // 2, g % 2
            pps = pspool.tile([128, SC], F32, tag="mm", name="pps")
            for hh, wt in enumerate((wo_t, wo2_t)):
                mm(pps[:],
                   sts[hh][:, sbl * 128:(sbl + 1) * 128],
                   wt[0:PD, pr * D + dc2 * SC:pr * D + (dc2 + 1) * SC],
                   start=(hh == 0), stop=(hh == 1))
            oev = opool.tile([128, SC], F16, tag="oev", name="oev", bufs=3)
            nc.vector.tensor_copy(oev[:], pps[:])
            nc.gpsimd.dma_start(
                dest[(row_base + sbl) * 128:(row_base + sbl + 1) * 128,
                     dc2 * SC:(dc2 + 1) * SC],
                oev[:])

        # ---- phase 1: qh chunk 0 + masks + v/k weights and first chunks ----
        xq0 = load_x(qp, 0)
        compute_qk(xq0, wq_t, qh, CST_BQ, 0)
        if causal:
            nc.sync.dma_start(msk_t[:, 0:2 * SC], msk[:, 0:2 * SC])
            nc.sync.dma_start(msk_t[:, 2 * SC:], msk[:, 2 * SC:])
        nc.sync.dma_start(wv_t[:, 0:NDC * HPD // 2], wv[:, 0:NDC * HPD // 2])
        nc.sync.dma_start(wv_t[:, NDC * HPD // 2:], wv[:, NDC * HPD // 2:])
        vls = {sb: load_v(sb) for sb in range(4)}
        nc.sync.dma_start(wk_t[:, 0:NDC * HPD // 2], wk[:, 0:NDC * HPD // 2])
        nc.sync.dma_start(wk_t[:, NDC * HPD // 2:], wk[:, NDC * HPD // 2:])
        xk0 = load_x(kp, 0)
        compute_v(vls.pop(0), 0)
        compute_v(vls.pop(1), 1)
        compute_qk(xk0, wk_t, kh, CST_BK, 0)

        xks = {}
        xqs = {}

        def Lv(sb):
            return lambda: vls.__setitem__(sb, load_v(sb))

        def Cv(sb):
            return lambda: compute_v(vls.pop(sb), sb)

        def Lk(c):
            return lambda: xks.__setitem__(c, load_x(kp, c))

        def Ck(c):
            return lambda: compute_qk(xks.pop(c), wk_t, kh, CST_BK, c)

        def Lq(c):
            return lambda: xqs.__setitem__(c, load_x(qp, c))

        def Cq(c):
            return lambda: compute_qk(xqs.pop(c), wq_t, qh, CST_BQ, c)

        # sweep(0,0): k chunks 1-3 and v blocks 2-15 as per-quad fill; loads
        # run one quad ahead of their consuming matmuls
        t00 = {
            0: [Cv(2), Cv(3), Lk(1), Lv(4), Lv(5), Lv(6), Lv(7)],
            1: [Ck(1), Cv(4), Cv(5), Cv(6), Cv(7), Lk(2), Lv(8), Lv(9),
                Lv(10), Lv(11)],
            2: [Ck(2), Cv(8), Cv(9), Cv(10), Cv(11), Lk(3), Lv(12), Lv(13),
                Lv(14), Lv(15),
                lambda: nc.sync.dma_start(wo_t[:], wo[:]),
                lambda: nc.sync.dma_start(wo2_t[:], wo[PD:128, :])],
            3: [Ck(3), Cv(12), Cv(13), Cv(14), Cv(15), Lq(1)],
        }
        avs0 = attention_pack(0, 0, tasks=t00)
        ohs0 = []
        normalize_pack(avs0, 0, ohs0)
        avs1 = attention_pack(0, 1, tasks={0: [Cq(1)], 2: [Lq(2)]})
        normalize_pack(avs1, 1, ohs0)

        # qc=1
        t10 = {1: [lambda: proj_out_group(0, ohs0, 0),
                   lambda: proj_out_group(0, ohs0, 1)],
               2: [lambda: proj_out_group(0, ohs0, 2), Cq(2)],
               3: [lambda: proj_out_group(0, ohs0, 3), Lq(3)]}
        avs0 = attention_pack(1, 0, tasks=t10)
        ohs1 = []
        normalize_pack(avs0, 0, ohs1)
        t11 = {0: [Cq(3)],
               1: [lambda: proj_out_group(0, ohs0, 4),
                   lambda: proj_out_group(0, ohs0, 5)],
               2: [lambda: proj_out_group(0, ohs0, 6)],
               3: [lambda: proj_out_group(0, ohs0, 7)]}
        avs1 = attention_pack(1, 1, tasks=t11)
        normalize_pack(avs1, 1, ohs1)

        # qc=2
        t20 = {1: [lambda: proj_out_group(1, ohs1, 0),
                   lambda: proj_out_group(1, ohs1, 1)],
               2: [lambda: proj_out_group(1, ohs1, 2),
                   lambda: proj_out_group(1, ohs1, 3)],
               3: [lambda: proj_out_group(1, ohs1, 4)]}
        avs0 = attention_pack(2, 0, tasks=t20)
        ohs2 = []
        normalize_pack(avs0, 0, ohs2)
        t21 = {1: [lambda: proj_out_group(1, ohs1, 5)],
               2: [lambda: proj_out_group(1, ohs1, 6)],
               3: [lambda: proj_out_group(1, ohs1, 7)]}
        avs1 = attention_pack(2, 1, tasks=t21)
        normalize_pack(avs1, 1, ohs2)

        # qc=3: pack0's half of the last out-projection runs inside pack1's
        # final sweep (into the out2 slab the host adds back); pack1's half
        # is the only post-sweep work
        t30 = {1: [lambda: proj_out_group(2, ohs2, 0),
                   lambda: proj_out_group(2, ohs2, 1)],
               2: [lambda: proj_out_group(2, ohs2, 2),
                   lambda: proj_out_group(2, ohs2, 3)],
               3: [lambda: proj_out_group(2, ohs2, 4)]}
        avs0 = attention_pack(3, 0, tasks=t30)
        sts0 = normalize_tail(avs0)
        t31 = {1: [lambda: proj_out_group(2, ohs2, 5),
                   lambda: proj_out_splitk(sts0, 0, 0, out2_d, 0)],
               2: [lambda: proj_out_group(2, ohs2, 6),
                   lambda: proj_out_splitk(sts0, 0, 1, out2_d, 0),
                   lambda: proj_out_splitk(sts0, 0, 2, out2_d, 0)],
               3: [lambda: proj_out_group(2, ohs2, 7),
                   lambda: proj_out_splitk(sts0, 0, 3, out2_d, 0),
                   lambda: proj_out_splitk(sts0, 0, 4, out2_d, 0)],
               4: [lambda: proj_out_splitk(sts0, 0, 5, out2_d, 0),
                   lambda: proj_out_splitk(sts0, 0, 6, out2_d, 0),
                   lambda: proj_out_splitk(sts0, 0, 7, out2_d, 0)]}
        avs1 = attention_pack(3, 1, tasks=t31)
        sts1 = normalize_tail(avs1)
        for g in range(8):
            proj_out_splitk(sts1, 1, g, out_d, 12)

    nc.compile()
    return nc


_programs = {}


def _get_program(causal: bool):
    if causal not in _programs:
        _programs[causal] = _build(causal)
    return _programs[causal]


def _make_cst(bq4, bk4, bv4):
    """Per-core fp32 constant blob [128, CST_COLS]."""
    cst = np.zeros((128, CST_COLS), np.float32)
    # per-pack per-partition biases: partition p of pack pk is d = pk*128+p
    cst[:, CST_BQ:CST_BQ + 2] = bq4.reshape(2, 128).T
    cst[:, CST_BK:CST_BK + 2] = bk4.reshape(2, 128).T
    # bv in free-dim layout [4*64], broadcast along partitions
    cst[:, CST_BV:CST_BV + HPD] = np.broadcast_to(bv4, (128, HPD))
    cst[:, CST_LOG2] = LOG2
    cst[:, CST_ZERO] = 0.0
    cst[:, CST_ONE] = 1.0
    return cst


def _make_mask(causal: bool) -> np.ndarray:
    """Diagonal-block additive log-masks [128, 4*SC]: log(2) iff
    q_local - 128*delta >= k_local (else 0); zeros when not causal."""
    m = np.zeros((128, NQD * SC), np.float32)
    if causal:
        kloc = np.arange(128)[:, None]
        qloc = np.arange(SC)[None, :]
        for delta in range(NQD):
            m[:, delta * SC:(delta + 1) * SC] = np.where(
                qloc - 128 * delta >= kloc, LOG2, 0.0)
    return m.astype(np.float16)


def _pack_xT(x):
    """[S, D] -> flat [128, NSC*NDC*SC] fp16: col ((sc*NDC)+c)*SC + s holds
    x[sc*SC+s, c*128+p]."""
    xT = np.ascontiguousarray(x.T, np.float16)          # [D, S]
    return np.ascontiguousarray(
        xT.reshape(NDC, 128, NSC, SC).transpose(1, 2, 0, 3).reshape(
            128, NSC * NDC * SC))


def _pack_vT(x):
    """[S, D] -> flat [128, NKB*NDC*128] fp16: col ((sb*NDC)+c)*128 + j holds
    x[sb*128+j, c*128+p]."""
    xT = np.ascontiguousarray(x.T, np.float16)          # [D, S]
    return np.ascontiguousarray(
        xT.reshape(NDC, 128, NKB, 128).transpose(1, 2, 0, 3).reshape(
            128, NKB * NDC * 128))


def _pack_w(w):
    """[D, HPD] -> flat [128, NDC*HPD] fp16: col c*HPD+m holds w[c*128+p, m]."""
    w16 = np.asarray(w, np.float16)
    return np.ascontiguousarray(
        w16.reshape(NDC, 128, HPD).transpose(1, 0, 2).reshape(128, NDC * HPD))


def _pack_wo(w):
    """[HPD, D] -> flat [128, NPACK*D] fp16: col r*D + n holds w[r*128+p, n]."""
    w16 = np.asarray(w, np.float16)
    return np.ascontiguousarray(
        w16.reshape(NPACK, 128, D).transpose(1, 0, 2).reshape(128, NPACK * D))


def kernel(**inputs) -> np.ndarray:
    q = np.asarray(inputs["q"], np.float32)
    k = np.asarray(inputs["k"], np.float32)
    v = np.asarray(inputs["v"], np.float32)
    Wq = np.asarray(inputs["Wq"], np.float32)
    Wk = np.asarray(inputs["Wk"], np.float32)
    Wv = np.asarray(inputs["Wv"], np.float32)
    Wo = np.asarray(inputs["Wo"], np.float32)
    bq = np.asarray(inputs["bq"], np.float32)
    bk = np.asarray(inputs["bk"], np.float32)
    bv = np.asarray(inputs["bv"], np.float32)
    bo = np.asarray(inputs["bo"], np.float32)
    causal = bool(np.asarray(inputs["use_causal_mask"]).item())

    nc = _get_program(causal)

    qpb = [_pack_xT(q[b]) for b in range(B)]
    kpb = [_pack_xT(k[b]) for b in range(B)]
    vpb = [_pack_vT(v[b]) for b in range(B)]
    mask = _make_mask(causal)

    in_maps = []
    for c in range(NCORES):
        b, hg = divmod(c, NCORES // B)
        cols = slice(hg * HPD, (hg + 1) * HPD)
        in_maps.append({
            "qp": qpb[b],
            "kp": kpb[b],
            "vp": vpb[b],
            "wq": _pack_w(Wq[:, cols]),
            "wk": _pack_w(Wk[:, cols]),
            "wv": _pack_w(Wv[:, cols]),
            "wo": _pack_wo(Wo[cols, :]),
            "cst": _make_cst(bq[cols], bk[cols], bv[cols]),
            "msk": mask,
        })

    res = run_bass_kernel_spmd(nc, in_maps, list(range(NCORES)))

    out = np.empty((B, S, D), np.float32)
    ncb = NCORES // B
    for b in range(B):
        acc = res.results[b * ncb]["out"].astype(np.float32)
        acc[(NSC - 1) * SC:] += res.results[b * ncb]["out2"].astype(np.float32)
        for c in range(b * ncb + 1, (b + 1) * ncb):
            acc += res.results[c]["out"].astype(np.float32)
            acc[(NSC - 1) * SC:] += res.results[c]["out2"].astype(np.float32)
        out[b] = acc + bo
    return out


# revision 9
# speedup vs baseline: 1.2043x; 1.2043x over previous
"""Trainium2 Bass kernel for nn_MultiHeadAttention (B=2, S=2048, D=1024, H=16).

Sharding: 8 cores; core c handles batch b=c//4 and the 4 heads
h in [4*(c%4), 4*(c%4)+4). Attention is embarrassingly parallel over (B, H);
the output projection is computed per-core over its head group (partial sums),
and the host sums the 4 partials per batch and adds the output bias.

All matmul operands are fp16 (10 mantissa bits — the same precision class as
tf32/float32r, measured end-to-end rel err ~1.5e-3) with fp32 PSUM
accumulation. fp16 halves every DMA against the serial ~360GB/s DMA-engine
resource and halves SBUF footprints vs fp32r at the same 1 row/cycle PE rate.

Per-core dataflow (contraction dim always on SBUF partitions):
  - host pre-packs q/k/v per batch into the exact per-partition SBUF layouts
    (flat [128, ...] slabs so every DMA descriptor is >=1KB: no
    small-descriptor 2x penalty) and converts to fp16
  - qh^T / kh^T [d, s] computed 2-heads-packed: head A on partitions 0-63,
    head B on 64-127 (lhsT = packed Wq columns, rhs = streamed xT chunks)
  - vh computed in natural [s, d] layout, with a ones-column appended -> the
    AV matmul also yields the softmax denominators
  - scores computed transposed s^T[k, q] so the softmax numerator
    exp(0.125*s + log2*causal) is produced by ScalarE directly in the
    AV-ready layout (k on partitions); no transposes needed anywhere on-chip.
    The reference's "mask" log(tril*1e-9 + 1e-9) is, by softmax shift
    invariance, exactly a x2 weight on the lower triangle.
  - scores/exp run in 4-key-block QUADS: one [128, 4, 512] psum tile per
    (head, quad), one ScalarE exp per quad. The dense sweeps are exp-paced
    (ScalarE is the secondary bottleneck at ~121us vs the PE's ~165us), so
    halving ScalarE's per-instruction PSUM-access overhead is critical.
  - AV: psum[65, 512] accumulates vh_aug.T @ e^T over 16 k-chunks; row 64 is
    the denominator. Normalize via DVE reciprocal + partition-broadcast.
  - out projection: head pairs stacked to K=128 (odd head hopped to
    partitions 64-127 over a SBUF->SBUF DMA); per (s-block, D-chunk) the two
    pair matmuls accumulate in psum; partial [S, D] DMAed out in fp16 on the
    Pool engine's SWDGE queue (keeps the SP queue free for input loads).
  - schedule: every projection matmul (k/v at qc=0, q chunk prefetch,
    out-projection groups) is spread as per-quad fill across BOTH packs'
    sweeps; DMA loads are issued 1-2 quads before their consuming matmuls so
    the in-order PE queue never waits on a same-slot DMA.
  - tail: the last q-chunk's out-projection is split by pack (pack0's half
    runs inside pack1's final sweep into a separate slab the host adds), and
    the pack halves use K=64 matmuls straight off the per-head normalize
    outputs — no partition-stack DMA on the critical path.
"""
import numpy as np
from contextlib import ExitStack

import concourse.bacc as bacc
import concourse.mybir as mybir
import concourse.tile as tile
from concourse.bass_utils import run_bass_kernel_spmd

F32 = mybir.dt.float32
F16 = mybir.dt.float16
AF = mybir.ActivationFunctionType
ALU = mybir.AluOpType

B, S, D, H, PD = 2, 2048, 1024, 16, 64
NCORES = 8
HPC = H * B // NCORES        # 4 heads per core
NPACK = HPC // 2             # 2 head-pairs per core
HPD = HPC * PD               # 256 projected columns per core
SC = 512                     # free-dim chunk (one fp32 psum bank)
NSC = S // SC                # 4
NKB = S // 128               # 16 key blocks / s blocks
NDC = D // 128               # 8 contraction chunks for the projections
NQD = 4                      # key-block quads per s-chunk
LOG2 = float(np.log(2.0))

# fp32 cst blob column layout (per partition)
CST_BQ = 0                   # [2] per-pack bq (per-partition scalars)
CST_BK = CST_BQ + 2          # [2]
CST_BV = CST_BK + 2          # [256] bv broadcast (free-dim layout)
CST_LOG2 = CST_BV + HPD      # [1] log(2) per partition (exp bias)
CST_ZERO = CST_LOG2 + 1      # [1] 0.0 per partition (exp bias)
CST_ONE = CST_ZERO + 1       # [1] 1.0 per partition
CST_COLS = CST_ONE + 1


def _build(causal: bool):
    nc = bacc.Bacc()
    qp = nc.dram_tensor("qp", [128, NSC * NDC * SC], F16, kind="ExternalInput")
    kp = nc.dram_tensor("kp", [128, NSC * NDC * SC], F16, kind="ExternalInput")
    vp = nc.dram_tensor("vp", [128, NKB * NDC * 128], F16,
                        kind="ExternalInput")
    wq = nc.dram_tensor("wq", [128, NDC * HPD], F16, kind="ExternalInput")
    wk = nc.dram_tensor("wk", [128, NDC * HPD], F16, kind="ExternalInput")
    wv = nc.dram_tensor("wv", [128, NDC * HPD], F16, kind="ExternalInput")
    wo = nc.dram_tensor("wo", [128, NPACK * D], F16, kind="ExternalInput")
    cst = nc.dram_tensor("cst", [128, CST_COLS], F32, kind="ExternalInput")
    msk = nc.dram_tensor("msk", [128, NQD * SC], F16, kind="ExternalInput")
    out_d = nc.dram_tensor("out", [S, D], F16, kind="ExternalOutput")
    out2_d = nc.dram_tensor("out2", [SC, D], F16, kind="ExternalOutput")

    mm = nc.tensor.matmul

    with tile.TileContext(nc) as tc, ExitStack() as ctx:
        cpool = ctx.enter_context(tc.tile_pool(name="cpool", bufs=1))
        xpool = ctx.enter_context(tc.tile_pool(name="xpool", bufs=2))
        hpool = ctx.enter_context(tc.tile_pool(name="hpool", bufs=1))
        epool = ctx.enter_context(tc.tile_pool(name="epool", bufs=3))
        opool = ctx.enter_context(tc.tile_pool(name="opool", bufs=2))
        spool = ctx.enter_context(tc.tile_pool(name="spool", bufs=2))
        pspool = ctx.enter_context(tc.tile_pool(name="ps", bufs=2,
                                                space="PSUM"))

        # ---- constants; HWDGE DMAs drain in emission order, so emit in
        # first-use order ----
        wq_t = cpool.tile([128, NDC * HPD], F16)
        nc.sync.dma_start(wq_t[:, 0:NDC * HPD // 2], wq[:, 0:NDC * HPD // 2])
        nc.sync.dma_start(wq_t[:, NDC * HPD // 2:], wq[:, NDC * HPD // 2:])
        cst_t = cpool.tile([128, CST_COLS], F32)
        nc.sync.dma_start(cst_t[:], cst[:])
        ones1 = cpool.tile([1, PD], F16)
        nc.vector.memset(ones1[:], 1.0)
        msk_t = cpool.tile([128, NQD * SC], F16)
        wk_t = cpool.tile([128, NDC * HPD], F16)
        wv_t = cpool.tile([128, NDC * HPD], F16)
        wo_t = cpool.tile([128, NPACK * D], F16)
        wo2_t = cpool.tile([PD, NPACK * D], F16)

        qh = [hpool.tile([128, S], F16, name=f"qh{p}") for p in range(NPACK)]
        kh = [hpool.tile([128, S], F16, name=f"kh{p}") for p in range(NPACK)]
        vh_all = hpool.tile([128, NKB, HPC, PD + 1], F16, name="vh_all")
        nc.vector.tensor_copy(
            vh_all[:, :, :, PD:PD + 1],
            cst_t[:, CST_ONE:CST_ONE + 1].to_broadcast((128, NKB, HPC, 1)))

        def load_x(xdram, sc):
            """DMA one [128, NDC*SC] s-chunk of packed q/k (4 descriptors)."""
            xTc = xpool.tile([128, NDC * SC], F16, tag="xTc", name="xTc",
                             bufs=3)
            w = NDC * SC // 4
            for i in range(4):
                nc.sync.dma_start(
                    xTc[:, i * w:(i + 1) * w],
                    xdram[:, sc * NDC * SC + i * w:sc * NDC * SC +
                          (i + 1) * w])
            return xTc

        def compute_qk(xTc, wtile, htiles, boff, sc):
            """Packed ^T projection matmuls for one loaded s-chunk."""
            for pk in range(NPACK):
                ps = pspool.tile([128, SC], F32, tag="mm", name="ps_qk")
                for dc in range(NDC):
                    mm(ps[:],
                       wtile[:, dc * HPD + pk * 128:
                             dc * HPD + (pk + 1) * 128],
                       xTc[:, dc * SC:(dc + 1) * SC],
                       start=(dc == 0), stop=(dc == NDC - 1))
                nc.vector.tensor_scalar(
                    htiles[pk][:, sc * SC:(sc + 1) * SC], ps[:],
                    cst_t[:, boff + pk: boff + pk + 1], None, ALU.add)

        bv_ap = cst_t[:, CST_BV: CST_BV + HPD].rearrange(
            "p (h d) -> p h d", h=HPC)

        def load_v(sb):
            vsl = xpool.tile([128, NDC * 128], F16, tag="vsl", name="vsl",
                             bufs=4)
            nc.sync.dma_start(
                vsl[:], vp[:, sb * NDC * 128:(sb + 1) * NDC * 128])
            return vsl

        def compute_v(vsl, sb):
            """One 128-row block of the natural-layout v projection."""
            ps = pspool.tile([128, HPD], F32, tag="mm", name="ps_v")
            for dc in range(NDC):
                mm(ps[:], vsl[:, dc * 128:(dc + 1) * 128],
                   wv_t[:, dc * HPD:(dc + 1) * HPD],
                   start=(dc == 0), stop=(dc == NDC - 1))
            nc.vector.tensor_tensor(
                vh_all[:, sb, :, 0:PD],
                ps[:].rearrange("p (h d) -> p h d", h=HPC),
                bv_ap,
                ALU.add)

        def score_exp_pair(qc, pk, hh, pair):
            """Scores^T for TWO consecutive k-blocks of one head into one
            2-bank psum tile, then a single [128, 2*SC] exp -> et2.

            Two psum tags (one per head) let ScalarE pipeline back-to-back;
            a single shared tile would serialize scores behind the other
            head's exp."""
            base = hh * PD
            sps = pspool.tile([128, 2, SC], F32, tag=f"s2h{hh}", name="sps",
                              bufs=1)
            for j in range(2):
                kb = 2 * pair + j
                mm(sps[:, j, :],
                   kh[pk][base:base + PD, kb * 128:(kb + 1) * 128],
                   qh[pk][base:base + PD, qc * SC:(qc + 1) * SC])
            et2 = epool.tile([128, 2, SC], F16, tag=f"e{hh}",
                             name=f"et{hh}", bufs=3)
            delta = 2 * pair - 4 * qc
            if causal and 0 <= delta < 4:
                # both k-blocks of the pair are diagonal blocks, and their
                # two mask tiles are adjacent msk columns
                tmp = epool.tile([128, 2, SC], F16, tag="tmp", name="tmp",
                                 bufs=2)
                moff = delta * SC
                nc.vector.scalar_tensor_tensor(
                    tmp[:], sps[:], 0.125,
                    msk_t[:, moff:moff + 2 * SC].rearrange(
                        "p (j s) -> p j s", j=2),
                    ALU.mult, ALU.add)
                nc.scalar.activation(
                    et2[:], tmp[:], AF.Exp,
                    bias=cst_t[:, CST_ZERO:CST_ZERO + 1], scale=1.0)
            else:
                boff = CST_LOG2 if (causal and delta < 0) else CST_ZERO
                nc.scalar.activation(
                    et2[:], sps[:], AF.Exp,
                    bias=cst_t[:, boff:boff + 1], scale=0.125)
            return et2

        def av_mm(av, i4, kb, et):
            mm(av[:], vh_all[:, kb, i4, :], et[:],
               start=(kb == 0), stop=(kb == NKB - 1))

        def attention_pack(qc, pk, tasks=None):
            """8 k-block-pair sweep for one pack (2 heads), AV one pair
            behind. Returns the pack's two [65, SC] psum accumulators."""
            avs = [pspool.tile([PD + 1, SC], F32, tag="av", name=f"av{hh}",
                               bufs=2)
                   for hh in range(2)]
            prevs = None
            npair = NKB // 2
            for pair in range(npair):
                if tasks is not None:
                    for t in tasks.get(pair, ()):
                        t()
                cur = [score_exp_pair(qc, pk, hh, pair) for hh in range(2)]
                if prevs is not None:
                    for hh in range(2):
                        for j in range(2):
                            av_mm(avs[hh], pk * 2 + hh,
                                  2 * (pair - 1) + j, prevs[hh][:, j, :])
                prevs = cur
            if tasks is not None:
                for t in tasks.get(npair, ()):
                    t()
            for hh in range(2):
                for j in range(2):
                    av_mm(avs[hh], pk * 2 + hh, NKB - 2 + j,
                          prevs[hh][:, j, :])
            return avs

        def normalize_pack(avs, pk, ohs):
            """outh^T = av[0:64] * bcast(1/av[64]) for the pack's 2 heads,
            stacked onto one [128, SC] tile (odd head hops to partitions
            64-127 over a SBUF->SBUF DMA) so the out-projection runs K=128.
            hh0 first: frees the next sweep's first av psum bank sooner."""
            oh = opool.tile([128, SC], F16, tag=f"ohp{pk}", name=f"ohp{pk}")
            ohs.append(oh)
            for hh in (0, 1):
                av = avs[hh]
                rrow = spool.tile([1, SC], F16, tag="rrow", name="rrow")
                with nc.allow_low_precision(
                        reason="fp16 1/denominator: 2^-11 rel, within budget"):
                    nc.vector.reciprocal(rrow[:], av[PD:PD + 1, :])
                rb = spool.tile([PD, SC], F16, tag="rb", name="rb")
                nc.sync.dma_start(
                    rb[:],
                    rrow[0:1, :].rearrange("p (o s) -> p o s",
                                           o=1).broadcast_to((1, PD, SC)))
                if hh == 0:
                    nc.vector.tensor_tensor(oh[0:PD, :], av[0:PD, :], rb[:],
                                            ALU.mult)
                else:
                    stg = spool.tile([PD, SC], F16, tag="stg", name="stg")
                    nc.vector.tensor_tensor(stg[:], av[0:PD, :], rb[:],
                                            ALU.mult)
                    nc.sync.dma_start(oh[PD:128, :], stg[:])

        def normalize_tail(avs):
            """Per-head normalize WITHOUT the partition-stack hop: returns the
            two [PD, SC] tiles. The reciprocal broadcast runs through a K=1
            PE matmul (no DMA-semaphore latency on the end-of-kernel path)."""
            sts = []
            rrows = []
            for hh in (0, 1):
                rrow = spool.tile([1, SC], F16, tag="rrow", name="rrow")
                with nc.allow_low_precision(
                        reason="fp16 1/denominator: 2^-11 rel, within budget"):
                    nc.vector.reciprocal(rrow[:], avs[hh][PD:PD + 1, :])
                rrows.append(rrow)
            for hh in (0, 1):
                bps = pspool.tile([PD, SC], F32, tag="mm", name="bps")
                mm(bps[:], ones1[:], rrows[hh][:])
                rb = spool.tile([PD, SC], F16, tag="rb", name="rb")
                nc.vector.tensor_copy(rb[:], bps[:])
                st = spool.tile([PD, SC], F16, tag="st", name=f"st{hh}",
                                bufs=4)
                nc.vector.tensor_tensor(st[:], avs[hh][0:PD, :], rb[:],
                                        ALU.mult)
                sts.append(st)
            return sts

        def proj_out_group(qc, ohs, g):
            """One stacked-K=128 out-projection group (s-block, D-chunk)."""
            sbl, dc2 = g // 2, g % 2
            sb = qc * 4 + sbl
            pps = pspool.tile([128, SC], F32, tag="mm", name="pps")
            for pr in range(NPACK):
                mm(pps[:],
                   ohs[pr][:, sbl * 128:(sbl + 1) * 128],
                   wo_t[:, pr * D + dc2 * SC:pr * D + (dc2 + 1) * SC],
                   start=(pr == 0), stop=(pr == NPACK - 1))
            oev = opool.tile([128, SC], F16, tag="oev", name="oev", bufs=3)
            nc.vector.tensor_copy(oev[:], pps[:])
            nc.gpsimd.dma_start(
                out_d[sb * 128:(sb + 1) * 128, dc2 * SC:(dc2 + 1) * SC],
                oev[:])

        def proj_out_splitk(sts, pr, g, dest, row_base):
            """One single-pack out-projection group as two K=64 matmuls
            straight off the normalize outputs (no stack hop)."""
            sbl, dc2 = g // 2, g % 2
            pps = pspool.tile([128, SC], F32, tag="mm", name="pps")
            for hh, wt in enumerate((wo_t, wo2_t)):
                mm(pps[:],
                   sts[hh][:, sbl * 128:(sbl + 1) * 128],
                   wt[0:PD, pr * D + dc2 * SC:pr * D + (dc2 + 1) * SC],
                   start=(hh == 0), stop=(hh == 1))
            oev = opool.tile([128, SC], F16, tag="oev", name="oev", bufs=3)
            nc.vector.tensor_copy(oev[:], pps[:])
            nc.gpsimd.dma_start(
                dest[(row_base + sbl) * 128:(row_base + sbl + 1) * 128,
                     dc2 * SC:(dc2 + 1) * SC],
                oev[:])

        # ---- phase 1: qh chunk 0 + masks + v/k weights and first chunks ----
        xq0 = load_x(qp, 0)
        compute_qk(xq0, wq_t, qh, CST_BQ, 0)
        if causal:
            nc.sync.dma_start(msk_t[:, 0:2 * SC], msk[:, 0:2 * SC])
            nc.sync.dma_start(msk_t[:, 2 * SC:], msk[:, 2 * SC:])
        nc.sync.dma_start(wv_t[:, 0:NDC * HPD // 2], wv[:, 0:NDC * HPD // 2])
        nc.sync.dma_start(wv_t[:, NDC * HPD // 2:], wv[:, NDC * HPD // 2:])
        vls = {sb: load_v(sb) for sb in range(4)}
        nc.sync.dma_start(wk_t[:, 0:NDC * HPD // 2], wk[:, 0:NDC * HPD // 2])
        nc.sync.dma_start(wk_t[:, NDC * HPD // 2:], wk[:, NDC * HPD // 2:])
        xk0 = load_x(kp, 0)
        compute_v(vls.pop(0), 0)
        compute_v(vls.pop(1), 1)
        compute_qk(xk0, wk_t, kh, CST_BK, 0)

        xks = {}
        xqs = {}

        def Lv(sb):
            return lambda: vls.__setitem__(sb, load_v(sb))

        def Cv(sb):
            return lambda: compute_v(vls.pop(sb), sb)

        def Lk(c):
            return lambda: xks.__setitem__(c, load_x(kp, c))

        def Ck(c):
            return lambda: compute_qk(xks.pop(c), wk_t, kh, CST_BK, c)

        def Lq(c):
            return lambda: xqs.__setitem__(c, load_x(qp, c))

        def Cq(c):
            return lambda: compute_qk(xqs.pop(c), wq_t, qh, CST_BQ, c)

        # sweep(0,0): k chunks 1-3 and v blocks 2-15 as per-quad fill; loads
        # run one quad ahead of their consuming matmuls
        t00 = {
            0: [Cv(2), Cv(3), Lk(1), Lv(4), Lv(5)],
            1: [Lv(6), Lv(7)],
            2: [Ck(1), Cv(4), Cv(5), Lk(2)],
            3: [Cv(6), Cv(7), Lv(8), Lv(9), Lv(10), Lv(11)],
            4: [Ck(2), Cv(8), Cv(9), Lk(3)],
            5: [Cv(10), Cv(11), Lv(12), Lv(13), Lv(14), Lv(15),
                lambda: nc.sync.dma_start(wo_t[:], wo[:]),
                lambda: nc.sync.dma_start(wo2_t[:], wo[PD:128, :])],
            6: [Ck(3), Cv(12), Cv(13), Lq(1)],
            7: [Cv(14), Cv(15)],
        }
        avs0 = attention_pack(0, 0, tasks=t00)
        ohs0 = []
        normalize_pack(avs0, 0, ohs0)
        avs1 = attention_pack(0, 1, tasks={0: [Cq(1)], 4: [Lq(2)]})
        normalize_pack(avs1, 1, ohs0)

        # qc=1
        t10 = {2: [lambda: proj_out_group(0, ohs0, 0)],
               3: [lambda: proj_out_group(0, ohs0, 1)],
               4: [lambda: proj_out_group(0, ohs0, 2), Cq(2)],
               5: [lambda: proj_out_group(0, ohs0, 3)],
               6: [Lq(3)]}
        avs0 = attention_pack(1, 0, tasks=t10)
        ohs1 = []
        normalize_pack(avs0, 0, ohs1)
        t11 = {0: [Cq(3)],
               2: [lambda: proj_out_group(0, ohs0, 4)],
               3: [lambda: proj_out_group(0, ohs0, 5)],
               4: [lambda: proj_out_group(0, ohs0, 6)],
               5: [lambda: proj_out_group(0, ohs0, 7)]}
        avs1 = attention_pack(1, 1, tasks=t11)
        normalize_pack(avs1, 1, ohs1)

        # qc=2
        t20 = {2: [lambda: proj_out_group(1, ohs1, 0)],
               3: [lambda: proj_out_group(1, ohs1, 1)],
               4: [lambda: proj_out_group(1, ohs1, 2)],
               5: [lambda: proj_out_group(1, ohs1, 3)],
               6: [lambda: proj_out_group(1, ohs1, 4)]}
        avs0 = attention_pack(2, 0, tasks=t20)
        ohs2 = []
        normalize_pack(avs0, 0, ohs2)
        t21 = {2: [lambda: proj_out_group(1, ohs1, 5)],
               3: [lambda: proj_out_group(1, ohs1, 6)],
               4: [lambda: proj_out_group(1, ohs1, 7)]}
        avs1 = attention_pack(2, 1, tasks=t21)
        normalize_pack(avs1, 1, ohs2)

        # qc=3: pack0's half of the last out-projection runs inside pack1's
        # final sweep (into the out2 slab the host adds back); pack1's half
        # is the only post-sweep work
        t30 = {2: [lambda: proj_out_group(2, ohs2, 0)],
               3: [lambda: proj_out_group(2, ohs2, 1)],
               4: [lambda: proj_out_group(2, ohs2, 2)],
               5: [lambda: proj_out_group(2, ohs2, 3)],
               6: [lambda: proj_out_group(2, ohs2, 4)]}
        avs0 = attention_pack(3, 0, tasks=t30)
        sts0 = normalize_tail(avs0)
        t31 = {2: [lambda: proj_out_group(2, ohs2, 5),
                   lambda: proj_out_splitk(sts0, 0, 0, out2_d, 0)],
               3: [lambda: proj_out_group(2, ohs2, 6),
                   lambda: proj_out_splitk(sts0, 0, 1, out2_d, 0)],
               4: [lambda: proj_out_group(2, ohs2, 7),
                   lambda: proj_out_splitk(sts0, 0, 2, out2_d, 0)],
               5: [lambda: proj_out_splitk(sts0, 0, 3, out2_d, 0),
                   lambda: proj_out_splitk(sts0, 0, 4, out2_d, 0)],
               6: [lambda: proj_out_splitk(sts0, 0, 5, out2_d, 0),
                   lambda: proj_out_splitk(sts0, 0, 6, out2_d, 0)],
               7: [lambda: proj_out_splitk(sts0, 0, 7, out2_d, 0)]}
        avs1 = attention_pack(3, 1, tasks=t31)
        sts1 = normalize_tail(avs1)
        for g in range(8):
            proj_out_splitk(sts1, 1, g, out_d, 12)

    nc.compile()
    return nc


_programs = {}


def _get_program(causal: bool):
    if causal not in _programs:
        _programs[causal] = _build(causal)
    return _programs[causal]


def _make_cst(bq4, bk4, bv4):
    """Per-core fp32 constant blob [128, CST_COLS]."""
    cst = np.zeros((128, CST_COLS), np.float32)
    # per-pack per-partition biases: partition p of pack pk is d = pk*128+p
    cst[:, CST_BQ:CST_BQ + 2] = bq4.reshape(2, 128).T
    cst[:, CST_BK:CST_BK + 2] = bk4.reshape(2, 128).T
    # bv in free-dim layout [4*64], broadcast along partitions
    cst[:, CST_BV:CST_BV + HPD] = np.broadcast_to(bv4, (128, HPD))
    cst[:, CST_LOG2] = LOG2
    cst[:, CST_ZERO] = 0.0
    cst[:, CST_ONE] = 1.0
    return cst


def _make_mask(causal: bool) -> np.ndarray:
    """Diagonal-block additive log-masks [128, 4*SC]: log(2) iff
    q_local - 128*delta >= k_local (else 0); zeros when not causal."""
    m = np.zeros((128, NQD * SC), np.float32)
    if causal:
        kloc = np.arange(128)[:, None]
        qloc = np.arange(SC)[None, :]
        for delta in range(NQD):
            m[:, delta * SC:(delta + 1) * SC] = np.where(
                qloc - 128 * delta >= kloc, LOG2, 0.0)
    return m.astype(np.float16)


def _pack_xT(x):
    """[S, D] -> flat [128, NSC*NDC*SC] fp16: col ((sc*NDC)+c)*SC + s holds
    x[sc*SC+s, c*128+p]."""
    xT = np.ascontiguousarray(x.T, np.float16)          # [D, S]
    return np.ascontiguousarray(
        xT.reshape(NDC, 128, NSC, SC).transpose(1, 2, 0, 3).reshape(
            128, NSC * NDC * SC))


def _pack_vT(x):
    """[S, D] -> flat [128, NKB*NDC*128] fp16: col ((sb*NDC)+c)*128 + j holds
    x[sb*128+j, c*128+p]."""
    xT = np.ascontiguousarray(x.T, np.float16)          # [D, S]
    return np.ascontiguousarray(
        xT.reshape(NDC, 128, NKB, 128).transpose(1, 2, 0, 3).reshape(
            128, NKB * NDC * 128))


def _pack_w(w):
    """[D, HPD] -> flat [128, NDC*HPD] fp16: col c*HPD+m holds w[c*128+p, m]."""
    w16 = np.asarray(w, np.float16)
    return np.ascontiguousarray(
        w16.reshape(NDC, 128, HPD).transpose(1, 0, 2).reshape(128, NDC * HPD))


def _pack_wo(w):
    """[HPD, D] -> flat [128, NPACK*D] fp16: col r*D + n holds w[r*128+p, n]."""
    w16 = np.asarray(w, np.float16)
    return np.ascontiguousarray(
        w16.reshape(NPACK, 128, D).transpose(1, 0, 2).reshape(128, NPACK * D))


def kernel(**inputs) -> np.ndarray:
    q = np.asarray(inputs["q"], np.float32)
    k = np.asarray(inputs["k"], np.float32)
    v = np.asarray(inputs["v"], np.float32)
    Wq = np.asarray(inputs["Wq"], np.float32)
    Wk = np.asarray(inputs["Wk"], np.float32)
    Wv = np.asarray(inputs["Wv"], np.float32)
    Wo = np.asarray(inputs["Wo"], np.float32)
    bq = np.asarray(inputs["bq"], np.float32)
    bk = np.asarray(inputs["bk"], np.float32)
    bv = np.asarray(inputs["bv"], np.float32)
    bo = np.asarray(inputs["bo"], np.float32)
    causal = bool(np.asarray(inputs["use_causal_mask"]).item())

    nc = _get_program(causal)

    qpb = [_pack_xT(q[b]) for b in range(B)]
    kpb = [_pack_xT(k[b]) for b in range(B)]
    vpb = [_pack_vT(v[b]) for b in range(B)]
    mask = _make_mask(causal)

    in_maps = []
    for c in range(NCORES):
        b, hg = divmod(c, NCORES // B)
        cols = slice(hg * HPD, (hg + 1) * HPD)
        in_maps.append({
            "qp": qpb[b],
            "kp": kpb[b],
            "vp": vpb[b],
            "wq": _pack_w(Wq[:, cols]),
            "wk": _pack_w(Wk[:, cols]),
            "wv": _pack_w(Wv[:, cols]),
            "wo": _pack_wo(Wo[cols, :]),
            "cst": _make_cst(bq[cols], bk[cols], bv[cols]),
            "msk": mask,
        })

    res = run_bass_kernel_spmd(nc, in_maps, list(range(NCORES)))

    out = np.empty((B, S, D), np.float32)
    ncb = NCORES // B
    for b in range(B):
        acc = res.results[b * ncb]["out"].astype(np.float32)
        acc[(NSC - 1) * SC:] += res.results[b * ncb]["out2"].astype(np.float32)
        for c in range(b * ncb + 1, (b + 1) * ncb):
            acc += res.results[c]["out"].astype(np.float32)
            acc[(NSC - 1) * SC:] += res.results[c]["out2"].astype(np.float32)
        out[b] = acc + bo
    return out


# revision 11
# speedup vs baseline: 1.2420x; 1.0313x over previous
"""Trainium2 Bass kernel for nn_MultiHeadAttention (B=2, S=2048, D=1024, H=16).

Sharding: 8 cores; core c handles batch b=c//4 and the 4 heads
h in [4*(c%4), 4*(c%4)+4). Attention is embarrassingly parallel over (B, H);
the output projection is computed per-core over its head group (partial sums),
and the host sums the 4 partials per batch and adds the output bias.

All matmul operands are fp16 (10 mantissa bits — the same precision class as
tf32/float32r, measured end-to-end rel err ~1.5e-3) with fp32 PSUM
accumulation. fp16 halves every DMA against the serial ~360GB/s DMA-engine
resource and halves SBUF footprints vs fp32r at the same 1 row/cycle PE rate.

Per-core dataflow (contraction dim always on SBUF partitions):
  - host pre-packs q/k/v per batch into the exact per-partition SBUF layouts
    (flat [128, ...] slabs so every DMA descriptor is >=1KB: no
    small-descriptor 2x penalty) and converts to fp16
  - qh^T / kh^T [d, s] computed 2-heads-packed: head A on partitions 0-63,
    head B on 64-127 (lhsT = packed Wq columns, rhs = streamed xT chunks)
  - vh computed in natural [s, d] layout, with a ones-column appended -> the
    AV matmul also yields the softmax denominators
  - scores computed transposed s^T[k, q] so the softmax numerator
    exp(0.125*s + log2*causal) is produced by ScalarE directly in the
    AV-ready layout (k on partitions); no transposes needed anywhere on-chip.
    The reference's "mask" log(tril*1e-9 + 1e-9) is, by softmax shift
    invariance, exactly a x2 weight on the lower triangle.
  - scores/exp run in 4-key-block QUADS: one [128, 4, 512] psum tile per
    (head, quad), one ScalarE exp per quad. The dense sweeps are exp-paced
    (ScalarE is the secondary bottleneck at ~121us vs the PE's ~165us), so
    halving ScalarE's per-instruction PSUM-access overhead is critical.
  - AV: psum[65, 512] accumulates vh_aug.T @ e^T over 16 k-chunks; row 64 is
    the denominator. Normalize via DVE reciprocal + partition-broadcast.
  - out projection: head pairs stacked to K=128 (odd head hopped to
    partitions 64-127 over a SBUF->SBUF DMA); per (s-block, D-chunk) the two
    pair matmuls accumulate in psum; partial [S, D] DMAed out in fp16 on the
    Pool engine's SWDGE queue (keeps the SP queue free for input loads).
  - schedule: every projection matmul (k/v at qc=0, q chunk prefetch,
    out-projection groups) is spread as per-quad fill across BOTH packs'
    sweeps; DMA loads are issued 1-2 quads before their consuming matmuls so
    the in-order PE queue never waits on a same-slot DMA.
  - tail: the last q-chunk's out-projection is split by pack (pack0's half
    runs inside pack1's final sweep into a separate slab the host adds), and
    the pack halves use K=64 matmuls straight off the per-head normalize
    outputs — no partition-stack DMA on the critical path.
"""
import numpy as np
from contextlib import ExitStack

import concourse.bacc as bacc
import concourse.mybir as mybir
import concourse.tile as tile
from concourse.bass_utils import run_bass_kernel_spmd

F32 = mybir.dt.float32
F16 = mybir.dt.float16
AF = mybir.ActivationFunctionType
ALU = mybir.AluOpType

B, S, D, H, PD = 2, 2048, 1024, 16, 64
NCORES = 8
HPC = H * B // NCORES        # 4 heads per core
NPACK = HPC // 2             # 2 head-pairs per core
HPD = HPC * PD               # 256 projected columns per core
SC = 512                     # free-dim chunk (one fp32 psum bank)
NSC = S // SC                # 4
NKB = S // 128               # 16 key blocks / s blocks
NDC = D // 128               # 8 contraction chunks for the projections
NQD = 4                      # key-block quads per s-chunk
LOG2 = float(np.log(2.0))

# fp32 cst blob column layout (per partition)
CST_BQ = 0                   # [2] per-pack bq (per-partition scalars)
CST_BK = CST_BQ + 2          # [2]
CST_BV = CST_BK + 2          # [256] bv broadcast (free-dim layout)
CST_LOG2 = CST_BV + HPD      # [1] log(2) per partition (exp bias)
CST_ZERO = CST_LOG2 + 1      # [1] 0.0 per partition (exp bias)
CST_ONE = CST_ZERO + 1       # [1] 1.0 per partition
CST_COLS = CST_ONE + 1


def _build(causal: bool):
    nc = bacc.Bacc()
    qp = nc.dram_tensor("qp", [128, NSC * NDC * SC], F16, kind="ExternalInput")
    kp = nc.dram_tensor("kp", [128, NSC * NDC * SC], F16, kind="ExternalInput")
    vp = nc.dram_tensor("vp", [128, NKB * NDC * 128], F16,
                        kind="ExternalInput")
    wq = nc.dram_tensor("wq", [128, NDC * HPD], F16, kind="ExternalInput")
    wk = nc.dram_tensor("wk", [128, NDC * HPD], F16, kind="ExternalInput")
    wv = nc.dram_tensor("wv", [128, NDC * HPD], F16, kind="ExternalInput")
    wo = nc.dram_tensor("wo", [128, NPACK * D], F16, kind="ExternalInput")
    cst = nc.dram_tensor("cst", [128, CST_COLS], F32, kind="ExternalInput")
    msk = nc.dram_tensor("msk", [128, NQD * SC], F16, kind="ExternalInput")
    out_d = nc.dram_tensor("out", [S, D], F16, kind="ExternalOutput")
    out2_d = nc.dram_tensor("out2", [SC, D], F16, kind="ExternalOutput")

    mm = nc.tensor.matmul

    with tile.TileContext(nc) as tc, ExitStack() as ctx:
        cpool = ctx.enter_context(tc.tile_pool(name="cpool", bufs=1))
        xpool = ctx.enter_context(tc.tile_pool(name="xpool", bufs=2))
        hpool = ctx.enter_context(tc.tile_pool(name="hpool", bufs=1))
        epool = ctx.enter_context(tc.tile_pool(name="epool", bufs=3))
        opool = ctx.enter_context(tc.tile_pool(name="opool", bufs=2))
        spool = ctx.enter_context(tc.tile_pool(name="spool", bufs=2))
        pspool = ctx.enter_context(tc.tile_pool(name="ps", bufs=2,
                                                space="PSUM"))

        # ---- constants; HWDGE DMAs drain in emission order, so emit in
        # first-use order ----
        wq_t = cpool.tile([128, NDC * HPD], F16)
        nc.sync.dma_start(wq_t[:, 0:NDC * HPD // 2], wq[:, 0:NDC * HPD // 2])
        nc.sync.dma_start(wq_t[:, NDC * HPD // 2:], wq[:, NDC * HPD // 2:])
        cst_t = cpool.tile([128, CST_COLS], F32)
        nc.sync.dma_start(cst_t[:], cst[:])
        ones1 = cpool.tile([1, PD], F16)
        nc.vector.memset(ones1[:], 1.0)
        msk_t = cpool.tile([128, NQD * SC], F16)
        wk_t = cpool.tile([128, NDC * HPD], F16)
        wv_t = cpool.tile([128, NDC * HPD], F16)
        wo_t = cpool.tile([128, NPACK * D], F16)
        wo2_t = cpool.tile([PD, NPACK * D], F16)

        qh = [hpool.tile([128, S], F16, name=f"qh{p}") for p in range(NPACK)]
        kh = [hpool.tile([128, S], F16, name=f"kh{p}") for p in range(NPACK)]
        vh_all = hpool.tile([128, NKB, HPC, PD + 1], F16, name="vh_all")
        nc.vector.tensor_copy(
            vh_all[:, :, :, PD:PD + 1],
            cst_t[:, CST_ONE:CST_ONE + 1].to_broadcast((128, NKB, HPC, 1)))

        def load_x(xdram, sc):
            """DMA one [128, NDC*SC] s-chunk of packed q/k (4 descriptors)."""
            xTc = xpool.tile([128, NDC * SC], F16, tag="xTc", name="xTc",
                             bufs=3)
            w = NDC * SC // 4
            for i in range(4):
                nc.sync.dma_start(
                    xTc[:, i * w:(i + 1) * w],
                    xdram[:, sc * NDC * SC + i * w:sc * NDC * SC +
                          (i + 1) * w])
            return xTc

        def compute_qk(xTc, wtile, htiles, boff, sc):
            """Packed ^T projection matmuls for one loaded s-chunk."""
            for pk in range(NPACK):
                ps = pspool.tile([128, SC], F32, tag="mm", name="ps_qk")
                for dc in range(NDC):
                    mm(ps[:],
                       wtile[:, dc * HPD + pk * 128:
                             dc * HPD + (pk + 1) * 128],
                       xTc[:, dc * SC:(dc + 1) * SC],
                       start=(dc == 0), stop=(dc == NDC - 1))
                nc.vector.tensor_scalar(
                    htiles[pk][:, sc * SC:(sc + 1) * SC], ps[:],
                    cst_t[:, boff + pk: boff + pk + 1], None, ALU.add)

        bv_ap = cst_t[:, CST_BV: CST_BV + HPD].rearrange(
            "p (h d) -> p h d", h=HPC)

        def load_v(sb):
            vsl = xpool.tile([128, NDC * 128], F16, tag="vsl", name="vsl",
                             bufs=4)
            nc.sync.dma_start(
                vsl[:], vp[:, sb * NDC * 128:(sb + 1) * NDC * 128])
            return vsl

        def compute_v(vsl, sb):
            """One 128-row block of the natural-layout v projection."""
            ps = pspool.tile([128, HPD], F32, tag="mm", name="ps_v")
            for dc in range(NDC):
                mm(ps[:], vsl[:, dc * 128:(dc + 1) * 128],
                   wv_t[:, dc * HPD:(dc + 1) * HPD],
                   start=(dc == 0), stop=(dc == NDC - 1))
            nc.vector.tensor_tensor(
                vh_all[:, sb, :, 0:PD],
                ps[:].rearrange("p (h d) -> p h d", h=HPC),
                bv_ap,
                ALU.add)

        def score_exp_pair(qc, pk, hh, pair):
            """Scores^T for TWO consecutive k-blocks of one head into one
            2-bank psum tile, then a single [128, 2*SC] exp -> et2.

            Two psum tags (one per head) let ScalarE pipeline back-to-back;
            a single shared tile would serialize scores behind the other
            head's exp."""
            base = hh * PD
            sps = pspool.tile([128, 2, SC], F32, tag=f"s2h{hh}", name="sps",
                              bufs=1)
            for j in range(2):
                kb = 2 * pair + j
                mm(sps[:, j, :],
                   kh[pk][base:base + PD, kb * 128:(kb + 1) * 128],
                   qh[pk][base:base + PD, qc * SC:(qc + 1) * SC])
            et2 = epool.tile([128, 2, SC], F16, tag=f"e{hh}",
                             name=f"et{hh}", bufs=3)
            delta = 2 * pair - 4 * qc
            if causal and 0 <= delta < 4:
                # both k-blocks of the pair are diagonal blocks, and their
                # two mask tiles are adjacent msk columns
                tmp = epool.tile([128, 2, SC], F16, tag="tmp", name="tmp",
                                 bufs=2)
                moff = delta * SC
                nc.vector.scalar_tensor_tensor(
                    tmp[:], sps[:], 0.125,
                    msk_t[:, moff:moff + 2 * SC].rearrange(
                        "p (j s) -> p j s", j=2),
                    ALU.mult, ALU.add)
                nc.scalar.activation(
                    et2[:], tmp[:], AF.Exp,
                    bias=cst_t[:, CST_ZERO:CST_ZERO + 1], scale=1.0)
            else:
                boff = CST_LOG2 if (causal and delta < 0) else CST_ZERO
                nc.scalar.activation(
                    et2[:], sps[:], AF.Exp,
                    bias=cst_t[:, boff:boff + 1], scale=0.125)
            return et2

        def av_mm(av, i4, kb, et):
            mm(av[:], vh_all[:, kb, i4, :], et[:],
               start=(kb == 0), stop=(kb == NKB - 1))

        def attention_pack(qc, pk, tasks=None):
            """8 k-block-pair sweep for one pack (2 heads), AV one pair
            behind. Returns the pack's two [65, SC] psum accumulators."""
            avs = [pspool.tile([PD + 1, SC], F32, tag="av", name=f"av{hh}",
                               bufs=2)
                   for hh in range(2)]
            prevs = None
            npair = NKB // 2
            for pair in range(npair):
                if tasks is not None:
                    for t in tasks.get(pair, ()):
                        t()
                cur = [score_exp_pair(qc, pk, hh, pair) for hh in range(2)]
                if prevs is not None:
                    for hh in range(2):
                        for j in range(2):
                            av_mm(avs[hh], pk * 2 + hh,
                                  2 * (pair - 1) + j, prevs[hh][:, j, :])
                prevs = cur
            if tasks is not None:
                for t in tasks.get(npair, ()):
                    t()
            for hh in range(2):
                for j in range(2):
                    av_mm(avs[hh], pk * 2 + hh, NKB - 2 + j,
                          prevs[hh][:, j, :])
            return avs

        def normalize_pack(avs, pk, ohs):
            """outh^T = av[0:64] * bcast(1/av[64]) for the pack's 2 heads,
            stacked onto one [128, SC] tile (odd head hops to partitions
            64-127 over a SBUF->SBUF DMA) so the out-projection runs K=128.
            hh0 first: frees the next sweep's first av psum bank sooner."""
            oh = opool.tile([128, SC], F16, tag=f"ohp{pk}", name=f"ohp{pk}")
            ohs.append(oh)
            for hh in (0, 1):
                av = avs[hh]
                rrow = spool.tile([1, SC], F16, tag="rrow", name="rrow")
                with nc.allow_low_precision(
                        reason="fp16 1/denominator: 2^-11 rel, within budget"):
                    nc.vector.reciprocal(rrow[:], av[PD:PD + 1, :])
                rb = spool.tile([PD, SC], F16, tag="rb", name="rb")
                nc.sync.dma_start(
                    rb[:],
                    rrow[0:1, :].rearrange("p (o s) -> p o s",
                                           o=1).broadcast_to((1, PD, SC)))
                if hh == 0:
                    nc.vector.tensor_tensor(oh[0:PD, :], av[0:PD, :], rb[:],
                                            ALU.mult)
                else:
                    stg = spool.tile([PD, SC], F16, tag="stg", name="stg")
                    nc.vector.tensor_tensor(stg[:], av[0:PD, :], rb[:],
                                            ALU.mult)
                    nc.sync.dma_start(oh[PD:128, :], stg[:])

        def normalize_tail(avs):
            """Per-head normalize WITHOUT the partition-stack hop: returns the
            two [PD, SC] tiles. The reciprocal broadcast runs through a K=1
            PE matmul (no DMA-semaphore latency on the end-of-kernel path)."""
            sts = []
            rrows = []
            for hh in (0, 1):
                rrow = spool.tile([1, SC], F16, tag="rrow", name="rrow")
                with nc.allow_low_precision(
                        reason="fp16 1/denominator: 2^-11 rel, within budget"):
                    nc.vector.reciprocal(rrow[:], avs[hh][PD:PD + 1, :])
                rrows.append(rrow)
            for hh in (0, 1):
                bps = pspool.tile([PD, SC], F32, tag="mm", name="bps")
                mm(bps[:], ones1[:], rrows[hh][:])
                rb = spool.tile([PD, SC], F16, tag="rb", name="rb")
                nc.vector.tensor_copy(rb[:], bps[:])
                st = spool.tile([PD, SC], F16, tag="st", name=f"st{hh}",
                                bufs=4)
                nc.vector.tensor_tensor(st[:], avs[hh][0:PD, :], rb[:],
                                        ALU.mult)
                sts.append(st)
            return sts

        def proj_out_group(qc, ohs, g):
            """One stacked-K=128 out-projection group (s-block, D-chunk)."""
            sbl, dc2 = g // 2, g % 2
            sb = qc * 4 + sbl
            pps = pspool.tile([128, SC], F32, tag="mm", name="pps")
            for pr in range(NPACK):
                mm(pps[:],
                   ohs[pr][:, sbl * 128:(sbl + 1) * 128],
                   wo_t[:, pr * D + dc2 * SC:pr * D + (dc2 + 1) * SC],
                   start=(pr == 0), stop=(pr == NPACK - 1))
            oev = opool.tile([128, SC], F16, tag="oev", name="oev", bufs=3)
            nc.vector.tensor_copy(oev[:], pps[:])
            nc.gpsimd.dma_start(
                out_d[sb * 128:(sb + 1) * 128, dc2 * SC:(dc2 + 1) * SC],
                oev[:])

        def proj_out_parts(qc, ohs, g):
            """A stacked out-projection group split into two single-matmul
            parts on different sweep pairs; part A (the settled pack0 oh) can
            land on the boundary pairs where pack1's oh is still in flight.
            The psum bank stays held between the parts (mm tag, bufs=2)."""
            sbl, dc2 = g // 2, g % 2
            sb = qc * 4 + sbl
            st = {}

            def part_a():
                st['pps'] = pspool.tile([128, SC], F32, tag="mm", name="pps")
                mm(st['pps'][:],
                   ohs[0][:, sbl * 128:(sbl + 1) * 128],
                   wo_t[:, dc2 * SC:(dc2 + 1) * SC],
                   start=True, stop=False)

            def part_b():
                pps = st['pps']
                mm(pps[:],
                   ohs[1][:, sbl * 128:(sbl + 1) * 128],
                   wo_t[:, D + dc2 * SC:D + (dc2 + 1) * SC],
                   start=False, stop=True)
                oev = opool.tile([128, SC], F16, tag="oev", name="oev",
                                 bufs=3)
                nc.vector.tensor_copy(oev[:], pps[:])
                nc.gpsimd.dma_start(
                    out_d[sb * 128:(sb + 1) * 128,
                          dc2 * SC:(dc2 + 1) * SC],
                    oev[:])

            return part_a, part_b

        def proj_out_splitk(sts, pr, g, dest, row_base, eng=None):
            """One single-pack out-projection group as two K=64 matmuls
            straight off the normalize outputs (no stack hop)."""
            sbl, dc2 = g // 2, g % 2
            pps = pspool.tile([128, SC], F32, tag="mm", name="pps")
            for hh, wt in enumerate((wo_t, wo2_t)):
                mm(pps[:],
                   sts[hh][:, sbl * 128:(sbl + 1) * 128],
                   wt[0:PD, pr * D + dc2 * SC:pr * D + (dc2 + 1) * SC],
                   start=(hh == 0), stop=(hh == 1))
            oev = opool.tile([128, SC], F16, tag="oev", name="oev", bufs=3)
            nc.vector.tensor_copy(oev[:], pps[:])
            (eng or nc.gpsimd).dma_start(
                dest[(row_base + sbl) * 128:(row_base + sbl + 1) * 128,
                     dc2 * SC:(dc2 + 1) * SC],
                oev[:])

        # ---- phase 1: qh chunk 0 + masks + v/k weights and first chunks ----
        xq0 = load_x(qp, 0)
        compute_qk(xq0, wq_t, qh, CST_BQ, 0)
        nc.sync.dma_start(wv_t[:, 0:NDC * HPD // 2], wv[:, 0:NDC * HPD // 2])
        nc.sync.dma_start(wv_t[:, NDC * HPD // 2:], wv[:, NDC * HPD // 2:])
        vls = {sb: load_v(sb) for sb in range(2)}
        nc.sync.dma_start(wk_t[:, 0:NDC * HPD // 2], wk[:, 0:NDC * HPD // 2])
        nc.sync.dma_start(wk_t[:, NDC * HPD // 2:], wk[:, NDC * HPD // 2:])
        xk0 = load_x(kp, 0)
        if causal:
            nc.sync.dma_start(msk_t[:, 0:2 * SC], msk[:, 0:2 * SC])
            nc.sync.dma_start(msk_t[:, 2 * SC:], msk[:, 2 * SC:])
        vls.update({sb: load_v(sb) for sb in (2, 3)})
        compute_v(vls.pop(0), 0)
        compute_v(vls.pop(1), 1)
        compute_qk(xk0, wk_t, kh, CST_BK, 0)

        xks = {}
        xqs = {}

        def Lv(sb):
            return lambda: vls.__setitem__(sb, load_v(sb))

        def Cv(sb):
            return lambda: compute_v(vls.pop(sb), sb)

        def Lk(c):
            return lambda: xks.__setitem__(c, load_x(kp, c))

        def Ck(c):
            return lambda: compute_qk(xks.pop(c), wk_t, kh, CST_BK, c)

        def Lq(c):
            return lambda: xqs.__setitem__(c, load_x(qp, c))

        def Cq(c):
            return lambda: compute_qk(xqs.pop(c), wq_t, qh, CST_BQ, c)

        # sweep(0,0): k chunks 1-3 and v blocks 2-15 as per-quad fill; loads
        # run one quad ahead of their consuming matmuls
        t00 = {
            0: [Cv(2), Cv(3), Lk(1), Lv(4), Lv(5)],
            1: [Lv(6), Lv(7)],
            2: [Ck(1), Cv(4), Cv(5), Lk(2)],
            3: [Cv(6), Cv(7), Lv(8), Lv(9), Lv(10), Lv(11)],
            4: [Ck(2), Cv(8), Cv(9), Lk(3)],
            5: [Cv(10), Cv(11), Lv(12), Lv(13), Lv(14), Lv(15),
                lambda: nc.sync.dma_start(wo_t[:], wo[:]),
                lambda: nc.sync.dma_start(wo2_t[:], wo[PD:128, :])],
            6: [Ck(3), Cv(12), Cv(13), Lq(1)],
            7: [Cv(14), Cv(15)],
        }
        avs0 = attention_pack(0, 0, tasks=t00)
        ohs0 = []
        normalize_pack(avs0, 0, ohs0)
        avs1 = attention_pack(0, 1, tasks={0: [Cq(1)], 4: [Lq(2)]})
        normalize_pack(avs1, 1, ohs0)

        def parts_sweep(qc, ohs):
            """(qc,0)-sweep fill map: 4 two-part groups; part A (settled
            pack0 oh) covers the boundary pairs, part B two pairs later."""
            ab = [proj_out_parts(qc, ohs, g) for g in range(4)]
            return {0: [ab[0][0]], 1: [ab[1][0]], 2: [ab[0][1]],
                    3: [ab[1][1]], 4: [ab[2][0]], 5: [ab[3][0]],
                    6: [ab[2][1]], 7: [ab[3][1]]}

        # qc=1
        avs0 = attention_pack(1, 0, tasks=parts_sweep(0, ohs0))
        ohs1 = []
        normalize_pack(avs0, 0, ohs1)
        t11 = {0: [lambda: proj_out_group(0, ohs0, 4)],
               1: [lambda: proj_out_group(0, ohs0, 5)],
               2: [Cq(2)],
               4: [lambda: proj_out_group(0, ohs0, 6)],
               5: [Lq(3)],
               6: [lambda: proj_out_group(0, ohs0, 7)]}
        avs1 = attention_pack(1, 1, tasks=t11)
        normalize_pack(avs1, 1, ohs1)

        # qc=2
        avs0 = attention_pack(2, 0, tasks=parts_sweep(1, ohs1))
        ohs2 = []
        normalize_pack(avs0, 0, ohs2)
        t21 = {0: [lambda: proj_out_group(1, ohs1, 4)],
               1: [lambda: proj_out_group(1, ohs1, 5)],
               2: [Cq(3)],
               4: [lambda: proj_out_group(1, ohs1, 6)],
               6: [lambda: proj_out_group(1, ohs1, 7)]}
        avs1 = attention_pack(2, 1, tasks=t21)
        normalize_pack(avs1, 1, ohs2)

        # qc=3: pack0's half of the last out-projection runs inside pack1's
        # final sweep (into the out2 slab the host adds back); pack1's half
        # is the only post-sweep work
        avs0 = attention_pack(3, 0, tasks=parts_sweep(2, ohs2))
        sts0 = normalize_tail(avs0)
        t31 = {0: [lambda: proj_out_group(2, ohs2, 4)],
               1: [lambda: proj_out_group(2, ohs2, 5)],
               2: [lambda: proj_out_splitk(sts0, 0, 0, out2_d, 0)],
               3: [lambda: proj_out_splitk(sts0, 0, 1, out2_d, 0)],
               4: [lambda: proj_out_group(2, ohs2, 6),
                   lambda: proj_out_splitk(sts0, 0, 2, out2_d, 0)],
               5: [lambda: proj_out_splitk(sts0, 0, 3, out2_d, 0),
                   lambda: proj_out_splitk(sts0, 0, 4, out2_d, 0)],
               6: [lambda: proj_out_group(2, ohs2, 7),
                   lambda: proj_out_splitk(sts0, 0, 5, out2_d, 0)],
               7: [lambda: proj_out_splitk(sts0, 0, 6, out2_d, 0),
                   lambda: proj_out_splitk(sts0, 0, 7, out2_d, 0)]}
        avs1 = attention_pack(3, 1, tasks=t31)
        sts1 = normalize_tail(avs1)
        for g in range(8):
            proj_out_splitk(sts1, 1, g, out_d, 12, eng=nc.sync)

    nc.compile()
    return nc


_programs = {}


def _get_program(causal: bool):
    if causal not in _programs:
        _programs[causal] = _build(causal)
    return _programs[causal]


def _make_cst(bq4, bk4, bv4):
    """Per-core fp32 constant blob [128, CST_COLS]."""
    cst = np.zeros((128, CST_COLS), np.float32)
    # per-pack per-partition biases: partition p of pack pk is d = pk*128+p
    cst[:, CST_BQ:CST_BQ + 2] = bq4.reshape(2, 128).T
    cst[:, CST_BK:CST_BK + 2] = bk4.reshape(2, 128).T
    # bv in free-dim layout [4*64], broadcast along partitions
    cst[:, CST_BV:CST_BV + HPD] = np.broadcast_to(bv4, (128, HPD))
    cst[:, CST_LOG2] = LOG2
    cst[:, CST_ZERO] = 0.0
    cst[:, CST_ONE] = 1.0
    return cst


def _make_mask(causal: bool) -> np.ndarray:
    """Diagonal-block additive log-masks [128, 4*SC]: log(2) iff
    q_local - 128*delta >= k_local (else 0); zeros when not causal."""
    m = np.zeros((128, NQD * SC), np.float32)
    if causal:
        kloc = np.arange(128)[:, None]
        qloc = np.arange(SC)[None, :]
        for delta in range(NQD):
            m[:, delta * SC:(delta + 1) * SC] = np.where(
                qloc - 128 * delta >= kloc, LOG2, 0.0)
    return m.astype(np.float16)


def _pack_xT(x):
    """[S, D] -> flat [128, NSC*NDC*SC] fp16: col ((sc*NDC)+c)*SC + s holds
    x[sc*SC+s, c*128+p]."""
    xT = np.ascontiguousarray(x.T, np.float16)          # [D, S]
    return np.ascontiguousarray(
        xT.reshape(NDC, 128, NSC, SC).transpose(1, 2, 0, 3).reshape(
            128, NSC * NDC * SC))


def _pack_vT(x):
    """[S, D] -> flat [128, NKB*NDC*128] fp16: col ((sb*NDC)+c)*128 + j holds
    x[sb*128+j, c*128+p]."""
    xT = np.ascontiguousarray(x.T, np.float16)          # [D, S]
    return np.ascontiguousarray(
        xT.reshape(NDC, 128, NKB, 128).transpose(1, 2, 0, 3).reshape(
            128, NKB * NDC * 128))


def _pack_w(w):
    """[D, HPD] -> flat [128, NDC*HPD] fp16: col c*HPD+m holds w[c*128+p, m]."""
    w16 = np.asarray(w, np.float16)
    return np.ascontiguousarray(
        w16.reshape(NDC, 128, HPD).transpose(1, 0, 2).reshape(128, NDC * HPD))


def _pack_wo(w):
    """[HPD, D] -> flat [128, NPACK*D] fp16: col r*D + n holds w[r*128+p, n]."""
    w16 = np.asarray(w, np.float16)
    return np.ascontiguousarray(
        w16.reshape(NPACK, 128, D).transpose(1, 0, 2).reshape(128, NPACK * D))


def kernel(**inputs) -> np.ndarray:
    q = np.asarray(inputs["q"], np.float32)
    k = np.asarray(inputs["k"], np.float32)
    v = np.asarray(inputs["v"], np.float32)
    Wq = np.asarray(inputs["Wq"], np.float32)
    Wk = np.asarray(inputs["Wk"], np.float32)
    Wv = np.asarray(inputs["Wv"], np.float32)
    Wo = np.asarray(inputs["Wo"], np.float32)
    bq = np.asarray(inputs["bq"], np.float32)
    bk = np.asarray(inputs["bk"], np.float32)
    bv = np.asarray(inputs["bv"], np.float32)
    bo = np.asarray(inputs["bo"], np.float32)
    causal = bool(np.asarray(inputs["use_causal_mask"]).item())

    nc = _get_program(causal)

    qpb = [_pack_xT(q[b]) for b in range(B)]
    kpb = [_pack_xT(k[b]) for b in range(B)]
    vpb = [_pack_vT(v[b]) for b in range(B)]
    mask = _make_mask(causal)

    in_maps = []
    for c in range(NCORES):
        b, hg = divmod(c, NCORES // B)
        cols = slice(hg * HPD, (hg + 1) * HPD)
        in_maps.append({
            "qp": qpb[b],
            "kp": kpb[b],
            "vp": vpb[b],
            "wq": _pack_w(Wq[:, cols]),
            "wk": _pack_w(Wk[:, cols]),
            "wv": _pack_w(Wv[:, cols]),
            "wo": _pack_wo(Wo[cols, :]),
            "cst": _make_cst(bq[cols], bk[cols], bv[cols]),
            "msk": mask,
        })

    res = run_bass_kernel_spmd(nc, in_maps, list(range(NCORES)))

    out = np.empty((B, S, D), np.float32)
    ncb = NCORES // B
    for b in range(B):
        acc = res.results[b * ncb]["out"].astype(np.float32)
        acc[(NSC - 1) * SC:] += res.results[b * ncb]["out2"].astype(np.float32)
        for c in range(b * ncb + 1, (b + 1) * ncb):
            acc += res.results[c]["out"].astype(np.float32)
            acc[(NSC - 1) * SC:] += res.results[c]["out2"].astype(np.float32)
        out[b] = acc + bo
    return out
